# revision 3
# baseline (speedup 1.0000x reference)
"""Recursive LSTM decoder (T=512, B=512, I=128, H=512) on 8 trn2 NeuronCores.

Strategy: data-parallel over batch (64 rows/core, weights replicated, no
collectives). All on-chip state is kept in transposed layout
[feature-on-partition, batch-on-free] so the serial recurrence needs no
transposes. Matmul inputs are bf16 (1 cycle/row on PE), accumulation and
elementwise math are fp32; the cell state c stays fp32.

Per step (per core):
  gates.T[m-chunk 128, b 64] = sum_k Wcat.T-chunk(k,m) @ catT-chunk(k)
    (16 m-chunks x 5 k-chunks; PSUM grouped by output H-chunk so ACT/DVE of
     group c overlaps PE of group c+1)
  i,f,o = sigmoid(. + b), g = tanh(. + b)   (bias folded into ACT)
  c = f*c + i*g ; h = o*tanh(c)
  feedback: inT = tanh(0.5*(fcW.T-chunks @ hT) + fc_b/2)   [= 2*sigmoid(z)-1]
  output:   out[64,128] = tanh(0.5*(hT-chunks as stationary @ fcW-moving + fc_b))
  out -> DRAM (fp16) at index (T-1-t)  (reference stores outputs reversed)

The loop trip count is a runtime input (niter), so one compiled NEFF serves
both the graded 512-step run and long timing runs (extra steps clamp their
store row to 0 and only matter for timing).

Host side: the PJRT executable is AOT-compiled once at import and cached at
module scope; per-call work is input prep + H2D (~23MB) + device exec + a
single fp16 D2H gather of the output.
"""

import numpy as np
import ml_dtypes

import concourse.bass as bass
import concourse.mybir as mybir
import concourse.tile as tile
from concourse import bacc
from concourse.bass import ds
from concourse.expressions import smax
from concourse._compat import axon_active

T, B, I, H = 512, 512, 128, 512
NCORES = 8
BS = B // NCORES          # 64 batch rows per core
HC = H // 128             # 4 h chunks
NM = (4 * H) // 128       # 16 gate m-chunks
NK = (I + H) // 128       # 5 cat k-chunks (1 input + 4 hidden)
MAX_STEPS = 1 << 22

# bf16 constant-bundle column offsets
OFF_WG = 0                       # [128, NM*NK*128] gate weight chunks
OFF_WFC = OFF_WG + NM * NK * 128  # [128, HC*128] fc weight chunks
OFF_XT = OFF_WFC + HC * 128      # [128, BS] x[T-1] transposed
OFF_H0 = OFF_XT + BS             # [128, HC*BS] h0 transposed
OFF_FCBR = OFF_H0 + HC * BS      # [1, 128] fc bias row (row 0 only)
CB_COLS = OFF_FCBR + 128
# f32 constant-bundle column offsets
OFF_BB = 0                       # [128, 4*HC*BS] gate bias broadcast (j,c,b)
OFF_FCBH = OFF_BB + 4 * HC * BS  # [128, 1] fc_b / 2
OFF_C0 = OFF_FCBH + 1            # [128, HC*BS] c0 transposed
CF_COLS = OFF_C0 + HC * BS

BF16 = mybir.dt.bfloat16
F32 = mybir.dt.float32
F16 = mybir.dt.float16
AF = mybir.ActivationFunctionType


def build(nsteps: int = T):
    nc = bacc.Bacc()
    cb16 = nc.dram_tensor("cb16", [128, CB_COLS], BF16, kind="ExternalInput")
    cf32 = nc.dram_tensor("cf32", [128, CF_COLS], F32, kind="ExternalInput")
    nit = nc.dram_tensor("niter", [1, 1], mybir.dt.uint32, kind="ExternalInput")
    out = nc.dram_tensor("out", [nsteps * BS, I], F16, kind="ExternalOutput")

    with tile.TileContext(nc) as tc:
        nregs = nc.alloc_registers("niter_regs")
        nc.regs_load(nregs, nit[0:1, 0:1])
        n_sv = nc.snap(nregs, donate=True, min_val=2, max_val=MAX_STEPS)

        with (
            tc.tile_pool(name="consts", bufs=1) as consts,
            tc.tile_pool(name="state", bufs=1) as state,
            tc.tile_pool(name="gact", bufs=3) as gact,
            tc.tile_pool(name="outp", bufs=3) as outp,
            tc.tile_pool(name="psst", bufs=1, space="PSUM") as psst,
            tc.tile_pool(name="pf", bufs=2, space="PSUM") as pfp,
            tc.tile_pool(name="po", bufs=2, space="PSUM") as pop,
        ):
            CB = consts.tile([128, CB_COLS], BF16)
            nc.sync.dma_start(out=CB, in_=cb16[:])
            CF = consts.tile([128, CF_COLS], F32)
            nc.sync.dma_start(out=CF, in_=cf32[:])
            ones = consts.tile([1, HC * BS], BF16)
            nc.vector.memset(ones, 1.0)

            def wg_chunk(m, k):
                o = OFF_WG + (m * NK + k) * 128
                return CB[:, o:o + 128]

            def wfc_chunk(k):
                o = OFF_WFC + k * 128
                return CB[:, o:o + 128]

            fb_r = CB[0:1, OFF_FCBR:OFF_FCBR + 128]
            BB = CF[:, OFF_BB:OFF_BB + 4 * HC * BS].rearrange(
                "p (j cb) -> p j cb", j=4)
            fb_h = CF[:, OFF_FCBH:OFF_FCBH + 1]

            hA = state.tile([128, HC, BS], BF16)
            nc.vector.tensor_copy(
                hA, CB[:, OFF_H0:OFF_H0 + HC * BS].rearrange(
                    "p (c b) -> p c b", c=HC))
            hB = state.tile([128, HC, BS], BF16)
            cT = state.tile([128, HC, BS], F32)
            nc.vector.tensor_copy(
                cT, CF[:, OFF_C0:OFF_C0 + HC * BS].rearrange(
                    "p (c b) -> p c b", c=HC))
            inT = state.tile([128, BS], BF16)
            nc.vector.tensor_copy(inT, CB[:, OFF_XT:OFF_XT + BS])
            # prologue tanh so the ACT table set is loaded on every path into
            # the loop -- otherwise the table-load lands INSIDE the body
            warm = state.tile([128, 1], F32)
            nc.scalar.activation(warm, CF[:, OFF_FCBH:OFF_FCBH + 1], AF.Tanh)

            # persistent per-gate PSUM accumulators [p, h-chunk, b]; prologue
            # dummy matmuls set every element's has_written bit so the
            # steady-state flow (DVE writes bias, matmuls accumulate with
            # start=False on top) works from the first step
            psg = [psst.tile([128, HC, BS], F32, name=f"psg{j}")
                   for j in range(4)]
            for j in range(4):
                nc.tensor.matmul(psg[j].rearrange("p c b -> p (c b)"),
                                 lhsT=ones[:, 0:128], rhs=ones,
                                 start=True, stop=True, skip_group_check=True)

            cTf = cT.rearrange("p c b -> p (c b)")
            psgf = [p.rearrange("p c b -> p (c b)") for p in psg]

            def step(t, h_in, h_out):
                # Per-gate PSUM: psg[j] holds gate j for all 4 H-chunks.
                # DVE pre-writes the bias into the bank; matmuls accumulate
                # on top (start=False, has_written set in prologue).
                # Gate order i, g, f, o so the c/h chain starts early.
                # sigmoid-free: sg(z)=(tanh(z/2)+1)/2, state C=2c, H=2h
                # (W_hh, fc_W host-halved; g-gate weights/bias host-doubled
                # so every gate uses tanh(0.5*psum)).
                th = {}
                for j in (0, 2, 1, 3):
                    nc.vector.tensor_copy(psgf[j], BB[:, j, :])
                    for c in range(HC):
                        m = j * 4 + c
                        for k in (1, 2, 3, 4, 0):
                            mv = inT if k == 0 else h_in[:, k - 1, :]
                            nc.tensor.matmul(
                                psg[j][:, c, :], lhsT=wg_chunk(m, k), rhs=mv,
                                start=False, stop=(k == 0),
                                skip_group_check=True)
                    th_j = gact.tile([128, HC * BS], F32, tag=f"th{j}")
                    nc.scalar.activation(th_j, psgf[j], AF.Tanh, scale=0.5)
                    th[j] = th_j
                # A=(th_f+1)*C=4fc, B=(th_i+1)*g=2ig, C_new=A/2+B=2c_new
                v_s = gact.tile([128, HC * BS], F32, tag="v_s")
                u_s = gact.tile([128, HC * BS], F32, tag="u_s")
                nc.vector.scalar_tensor_tensor(
                    v_s, th[0], 1.0, th[2],
                    op0=mybir.AluOpType.add, op1=mybir.AluOpType.mult)
                nc.vector.scalar_tensor_tensor(
                    u_s, th[1], 1.0, cTf,
                    op0=mybir.AluOpType.add, op1=mybir.AluOpType.mult)
                nc.vector.scalar_tensor_tensor(
                    cTf, u_s, 0.5, v_s,
                    op0=mybir.AluOpType.mult, op1=mybir.AluOpType.add)
                tc_s = gact.tile([128, HC * BS], F32, tag="tc_s")
                nc.scalar.activation(tc_s, cTf, AF.Tanh, scale=0.5)
                # H = (th_o+1)*tanh(c) = 2h
                nc.vector.scalar_tensor_tensor(
                    h_out.rearrange("p c b -> p (c b)"), th[3], 1.0, tc_s,
                    op0=mybir.AluOpType.add, op1=mybir.AluOpType.mult)

                # feedback fc: inT = tanh(0.5*fc(h) + fc_b/2)  [128 i, BS b]
                pf = pfp.tile([128, BS], F32, tag="pf")
                for k in range(HC):
                    nc.tensor.matmul(pf, lhsT=wfc_chunk(k), rhs=h_out[:, k, :],
                                     start=(k == 0), stop=(k == HC - 1))
                nc.scalar.activation(inT, pf, AF.Tanh, bias=fb_h, scale=0.5)

                # output fc in [b, i] layout for clean DMA; bias via K=1 matmul
                po = pop.tile([BS, 128], F32, tag="po")
                for k in range(HC):
                    nc.tensor.matmul(po, lhsT=h_out[:, k, :], rhs=wfc_chunk(k),
                                     start=(k == 0), stop=False)
                nc.tensor.matmul(po, lhsT=ones[:, 0:BS], rhs=fb_r,
                                 start=False, stop=True)
                ob = outp.tile([BS, 128], F16, tag="ob")
                nc.scalar.activation(ob, po, AF.Tanh, scale=0.5)
                # steps past nsteps (timing mode) clamp to row 0 (junk)
                row = smax(0, (nsteps - 1 - t) * BS)
                nc.sync.dma_start(out=out[ds(row, BS), :], in_=ob)

            with tc.For_i(0, n_sv, 2, staggered_reset=True,
                          hint_engines=(mybir.EngineType.PE,)) as t:
                step(t, hA, hB)
                step(t + 1, hB, hA)

    nc.finalize()
    return nc


def _prep_core_inputs(x, h0, c0, W_ih, W_hh, b_ih, b_hh, fc_W, fc_b,
                      nsteps=T, niter=T):
    f32 = np.float32
    bf16 = ml_dtypes.bfloat16
    x = np.asarray(x, f32)
    h0 = np.asarray(h0, f32)
    c0 = np.asarray(c0, f32)
    # state is H=2h, C=2c with W_hh/fc_W halved to compensate; g-gate rows
    # doubled so all gates share tanh(0.5*(psum)) with psum pre-biased
    W_cat = np.concatenate(
        [np.asarray(W_ih, f32), 0.5 * np.asarray(W_hh, f32)], axis=1)
    W_cat[1024:1536, :] *= 2.0
    wg_np = W_cat.reshape(NM, 128, NK, 128).transpose(3, 0, 2, 1).reshape(
        128, NM * NK * 128)
    fc_W = np.asarray(fc_W, f32)
    wfc_np = (0.5 * fc_W).reshape(I, HC, 128).transpose(2, 1, 0).reshape(
        128, HC * 128)
    b = np.asarray(b_ih, f32) + np.asarray(b_hh, f32)
    badj = b.copy()
    badj[1024:1536] *= 2.0
    # bias broadcast [p, gate j, h-chunk c, b] -> [128, 4*HC*BS]
    bb_np = np.broadcast_to(
        badj.reshape(4, HC, 128).transpose(2, 0, 1)[:, :, :, None],
        (128, 4, HC, BS)).reshape(128, 4 * HC * BS)
    fc_b = np.asarray(fc_b, f32)

    cf = np.zeros((128, CF_COLS), f32)
    cf[:, OFF_BB:OFF_BB + 4 * HC * BS] = bb_np
    cf[:, OFF_FCBH] = 0.5 * fc_b

    cb_common = np.zeros((128, CB_COLS), f32)
    cb_common[:, OFF_WG:OFF_WG + NM * NK * 128] = wg_np
    cb_common[:, OFF_WFC:OFF_WFC + HC * 128] = wfc_np
    cb_common[0, OFF_FCBR:OFF_FCBR + 128] = fc_b

    nit = np.full((1, 1), niter, np.uint32)
    in_maps = []
    for core in range(NCORES):
        sl = slice(core * BS, (core + 1) * BS)
        cb = cb_common.copy()
        cb[:, OFF_XT:OFF_XT + BS] = x[nsteps - 1, sl, :].T
        cb[:, OFF_H0:OFF_H0 + HC * BS] = 2.0 * \
            h0[0, sl, :].reshape(BS, HC, 128).transpose(2, 1, 0).reshape(128, -1)
        cfc = cf.copy()
        cfc[:, OFF_C0:OFF_C0 + HC * BS] = 2.0 * \
            c0[0, sl, :].reshape(BS, HC, 128).transpose(2, 1, 0).reshape(128, -1)
        in_maps.append({
            "cb16": np.ascontiguousarray(cb).astype(bf16),
            "cf32": np.ascontiguousarray(cfc),
            "niter": nit,
        })
    return in_maps


# ---------------------------------------------------------------------------
# PJRT runner: AOT-compiled once, executed per call. Only used under axon
# (the graded environment); native TRN2 falls back to run_bass_kernel_spmd.
# ---------------------------------------------------------------------------

_STATE = {}


def _init_runner():
    if "runner" in _STATE:
        return _STATE["runner"]
    import jax
    import jax.numpy as jnp
    from jax.sharding import Mesh, PartitionSpec, NamedSharding
    import warnings
    with warnings.catch_warnings():
        warnings.simplefilter("ignore")
        from jax.experimental.shard_map import shard_map
    from concourse.bass2jax import (
        _bass_exec_p, install_neuronx_cc_hook, partition_id_tensor)

    install_neuronx_cc_hook()
    nc = _STATE.get("nc")
    if nc is None:
        nc = _STATE["nc"] = build(T)

    partition_name = (nc.partition_id_tensor.name
                      if nc.partition_id_tensor else None)
    in_names, out_names, out_avals = [], [], []
    for alloc in nc.m.functions[0].allocations:
        if not isinstance(alloc, mybir.MemoryLocationSet):
            continue
        name = alloc.memorylocations[0].name
        if alloc.kind == "ExternalInput":
            if name != partition_name:
                in_names.append(name)
        elif alloc.kind == "ExternalOutput":
            out_names.append(name)
            out_avals.append(jax.core.ShapedArray(
                tuple(alloc.tensor_shape), mybir.dt.np(alloc.dtype)))
    n_params = len(in_names)
    n_outs = len(out_avals)
    in_names_all = in_names + out_names + (
        [partition_name] if partition_name else [])

    def _body(*args):
        operands = list(args)
        if partition_name:
            operands.append(partition_id_tensor())
        outs = _bass_exec_p.bind(
            *operands, out_avals=tuple(out_avals),
            in_names=tuple(in_names_all), out_names=tuple(out_names),
            lowering_input_output_aliases=(), sim_require_finite=True,
            sim_require_nnan=True, nc=nc)
        return tuple(outs)

    devs = jax.devices()[:NCORES]
    mesh = Mesh(np.asarray(devs), ("core",))
    donate = tuple(range(n_params, n_params + n_outs))
    sharded = jax.jit(
        shard_map(_body, mesh=mesh,
                  in_specs=(PartitionSpec("core"),) * (n_params + n_outs),
                  out_specs=(PartitionSpec("core"),) * n_outs,
                  check_rep=False),
        donate_argnums=donate, keep_unused=True)

    in_shapes = {
        "cb16": ((NCORES * 128, CB_COLS), ml_dtypes.bfloat16),
        "cf32": ((NCORES * 128, CF_COLS), np.float32),
        "niter": ((NCORES * 1, 1), np.uint32),
    }
    out_shapes = [((NCORES * a.shape[0],) + tuple(a.shape[1:]), a.dtype)
                  for a in out_avals]
    abstract = ([jax.ShapeDtypeStruct(*in_shapes[nm]) for nm in in_names]
                + [jax.ShapeDtypeStruct(s, d) for s, d in out_shapes])
    compiled = sharded.lower(*abstract).compile()

    out_sharding = NamedSharding(mesh, PartitionSpec("core"))
    zeros_fns = [
        jax.jit(lambda s=s, d=d: jnp.zeros(s, d), out_shardings=out_sharding)
        for s, d in out_shapes]

    runner = _STATE["runner"] = {
        "jax": jax, "compiled": compiled, "in_names": in_names,
        "zeros_fns": zeros_fns, "n_outs": n_outs,
    }
    return runner


def _run_axon(in_maps):
    r = _init_runner()
    jax = r["jax"]
    concat_in = [np.concatenate([m[nm] for m in in_maps], axis=0)
                 for nm in r["in_names"]]
    zeros = [fn() for fn in r["zeros_fns"]]
    outs = r["compiled"](*concat_in, *zeros)
    res = np.asarray(outs[0])  # fp16 [NCORES*T*BS, I], one D2H gather
    return res


def warmup():
    """Compile + one tiny dummy exec so later calls are transfer+exec only."""
    if _STATE.get("warm"):
        return
    try:
        r = _init_runner()
        dummy = [{"cb16": np.zeros((128, CB_COLS), ml_dtypes.bfloat16),
                  "cf32": np.zeros((128, CF_COLS), np.float32),
                  "niter": np.full((1, 1), 2, np.uint32)}
                 for _ in range(NCORES)]
        res = _run_axon(dummy)
        del res
        _STATE["warm"] = True
    except Exception:
        import traceback
        traceback.print_exc()


def run(x, h0, c0, W_ih, W_hh, b_ih, b_hh, fc_W, fc_b, niter=T, **kwargs):
    """Returns fp32 [T, B, I] output (valid when niter == T)."""
    in_maps = _prep_core_inputs(x, h0, c0, W_ih, W_hh, b_ih, b_hh, fc_W, fc_b,
                                nsteps=T, niter=niter)
    if axon_active():
        res = _run_axon(in_maps)
        per_core = res.reshape(NCORES, T, BS, I)
    else:
        from concourse.bass_utils import run_bass_kernel_spmd
        nc = _STATE.get("nc")
        if nc is None:
            nc = _STATE["nc"] = build(T)
        rr = run_bass_kernel_spmd(nc, in_maps, core_ids=list(range(NCORES)),
                                  **kwargs)
        per_core = np.stack(
            [r["out"].reshape(T, BS, I) for r in rr.results], axis=0)
    out = np.empty((T, B, I), np.float32)
    for c in range(NCORES):
        out[:, c * BS:(c + 1) * BS, :] = per_core[c]
    return out


def kernel(x, enc_hiddens, h0, c0, W_ih, W_hh, b_ih, b_hh, fc_W, fc_b):
    return run(x, h0, c0, W_ih, W_hh, b_ih, b_hh, fc_W, fc_b)


if axon_active():
    warmup()


# revision 9
# speedup vs baseline: 27.8800x; 27.8800x over previous
"""Recursive LSTM decoder (T=512, B=512, I=128, H=512) on 8 trn2 NeuronCores.

Strategy: data-parallel over batch (64 rows/core, weights replicated, no
collectives). All on-chip state is kept in transposed layout
[feature-on-partition, batch-on-free] so the serial recurrence needs no
transposes. Matmul inputs are bf16 (1 cycle/row on PE), accumulation and
elementwise math are fp32; the cell state c stays fp32.

Per step (per core):
  gates.T[m-chunk 128, b 64] = sum_k Wcat.T-chunk(k,m) @ catT-chunk(k)
    (16 m-chunks x 5 k-chunks; PSUM grouped by output H-chunk so ACT/DVE of
     group c overlaps PE of group c+1)
  i,f,o = sigmoid(. + b), g = tanh(. + b)   (bias folded into ACT)
  c = f*c + i*g ; h = o*tanh(c)
  feedback: inT = tanh(0.5*(fcW.T-chunks @ hT) + fc_b/2)   [= 2*sigmoid(z)-1]
  output:   out[64,128] = tanh(0.5*(hT-chunks as stationary @ fcW-moving + fc_b))
  out -> DRAM (fp16) at index (T-1-t)  (reference stores outputs reversed)

The loop trip count is a runtime input (niter), so one compiled NEFF serves
both the graded 512-step run and long timing runs (extra steps clamp their
store row to 0 and only matter for timing).

Host side: the PJRT executable is AOT-compiled once at import and cached at
module scope; per-call work is input prep + H2D (~23MB) + device exec + a
single fp16 D2H gather of the output.
"""

import numpy as np
import ml_dtypes

import concourse.bass as bass
import concourse.mybir as mybir
import concourse.tile as tile
from concourse import bacc
from concourse.bass import ds
from concourse.expressions import smax
from concourse._compat import axon_active

T, B, I, H = 512, 512, 128, 512
NCORES = 8
BS = B // NCORES          # 64 batch rows per core
HC = H // 128             # 4 h chunks
NM = (4 * H) // 128       # 16 gate m-chunks
NK = (I + H) // 128       # 5 cat k-chunks (1 input + 4 hidden)
MAX_STEPS = 1 << 22

# bf16 constant-bundle column offsets
OFF_WG = 0                       # [128, NM*NK*128] gate weight chunks
OFF_WFC = OFF_WG + NM * NK * 128  # [128, HC*128] fc weight chunks
OFF_XT = OFF_WFC + HC * 128      # [128, BS] x[T-1] transposed
OFF_H0 = OFF_XT + BS             # [128, HC*BS] h0 transposed
OFF_FCBR = OFF_H0 + HC * BS      # [1, 128] fc bias row (row 0 only)
CB_COLS = OFF_FCBR + 128
# f32 constant-bundle column offsets
OFF_BB = 0                       # [128, 4*HC*BS] gate bias broadcast (j,c,b)
OFF_FCBH = OFF_BB + 4 * HC * BS  # [128, 1] fc_b / 2
OFF_C0 = OFF_FCBH + 1            # [128, HC*BS] c0 transposed
CF_COLS = OFF_C0 + HC * BS

BF16 = mybir.dt.bfloat16
F32 = mybir.dt.float32
F16 = mybir.dt.float16
AF = mybir.ActivationFunctionType


def build(nsteps: int = T, unroll: int = 2, staggered: bool = True,
          hints: tuple = ("PE",)):
    nc = bacc.Bacc()
    cb16 = nc.dram_tensor("cb16", [128, CB_COLS], BF16, kind="ExternalInput")
    cf32 = nc.dram_tensor("cf32", [128, CF_COLS], F32, kind="ExternalInput")
    nit = nc.dram_tensor("niter", [1, 1], mybir.dt.uint32, kind="ExternalInput")
    out = nc.dram_tensor("out", [nsteps * BS, I], F16, kind="ExternalOutput")

    with tile.TileContext(nc) as tc:
        nregs = nc.alloc_registers("niter_regs")
        nc.regs_load(nregs, nit[0:1, 0:1])
        n_sv = nc.snap(nregs, donate=True, min_val=2, max_val=MAX_STEPS)

        with (
            tc.tile_pool(name="consts", bufs=1) as consts,
            tc.tile_pool(name="state", bufs=1) as state,
            tc.tile_pool(name="gact", bufs=3) as gact,
            tc.tile_pool(name="outp", bufs=3) as outp,
            tc.tile_pool(name="psst", bufs=1, space="PSUM") as psst,
            tc.tile_pool(name="pf", bufs=2, space="PSUM") as pfp,
            tc.tile_pool(name="po", bufs=2, space="PSUM") as pop,
        ):
            CB = consts.tile([128, CB_COLS], BF16)
            nc.sync.dma_start(out=CB, in_=cb16[:])
            CF = consts.tile([128, CF_COLS], F32)
            nc.sync.dma_start(out=CF, in_=cf32[:])
            ones = consts.tile([1, HC * BS], BF16)
            nc.vector.memset(ones, 1.0)

            def wg_chunk(m, k):
                o = OFF_WG + (m * NK + k) * 128
                return CB[:, o:o + 128]

            def wfc_chunk(k):
                o = OFF_WFC + k * 128
                return CB[:, o:o + 128]

            fb_r = CB[0:1, OFF_FCBR:OFF_FCBR + 128]
            BB = CF[:, OFF_BB:OFF_BB + 4 * HC * BS].rearrange(
                "p (j cb) -> p j cb", j=4)
            fb_h = CF[:, OFF_FCBH:OFF_FCBH + 1]

            hA = state.tile([128, HC, BS], BF16)
            nc.vector.tensor_copy(
                hA, CB[:, OFF_H0:OFF_H0 + HC * BS].rearrange(
                    "p (c b) -> p c b", c=HC))
            hB = state.tile([128, HC, BS], BF16)
            cT = state.tile([128, HC, BS], F32)
            nc.vector.tensor_copy(
                cT, CF[:, OFF_C0:OFF_C0 + HC * BS].rearrange(
                    "p (c b) -> p c b", c=HC))
            inT = state.tile([128, BS], BF16)
            nc.vector.tensor_copy(inT, CB[:, OFF_XT:OFF_XT + BS])
            # prologue tanh so the ACT table set is loaded on every path into
            # the loop -- otherwise the table-load lands INSIDE the body
            warm = state.tile([128, 1], F32)
            nc.scalar.activation(warm, CF[:, OFF_FCBH:OFF_FCBH + 1], AF.Tanh)

            # persistent per-gate PSUM accumulators [p, h-chunk, b]; prologue
            # dummy matmuls set every element's has_written bit so the
            # steady-state flow (DVE writes bias, matmuls accumulate with
            # start=False on top) works from the first step
            psg = [psst.tile([128, HC, BS], F32, name=f"psg{j}")
                   for j in range(4)]
            for j in range(4):
                nc.tensor.matmul(psg[j].rearrange("p c b -> p (c b)"),
                                 lhsT=ones[:, 0:128], rhs=ones,
                                 start=True, stop=True, skip_group_check=True)

            cTf = cT.rearrange("p c b -> p (c b)")
            psgf = [p.rearrange("p c b -> p (c b)") for p in psg]

            def step(t, h_in, h_out):
                # Per-gate PSUM: psg[j] holds gate j for all 4 H-chunks.
                # DVE pre-writes the bias into the bank; matmuls accumulate
                # on top (start=False, has_written set in prologue).
                # Gate order i, g, f, o so the c/h chain starts early.
                # sigmoid-free: sg(z)=(tanh(z/2)+1)/2, state C=2c, H=2h
                # (W_hh, fc_W host-halved; g-gate weights/bias host-doubled
                # so every gate uses tanh(0.5*psum)).
                th = {}
                for j in (0, 2, 1, 3):
                    nc.vector.tensor_copy(psgf[j], BB[:, j, :])
                    for c in range(HC):
                        m = j * 4 + c
                        for k in (1, 2, 3, 4, 0):
                            mv = inT if k == 0 else h_in[:, k - 1, :]
                            nc.tensor.matmul(
                                psg[j][:, c, :], lhsT=wg_chunk(m, k), rhs=mv,
                                start=False, stop=(k == 0),
                                skip_group_check=True)
                    th_j = gact.tile([128, HC * BS], F32, tag=f"th{j}")
                    nc.scalar.activation(th_j, psgf[j], AF.Tanh, scale=0.5)
                    th[j] = th_j
                # A=(th_f+1)*C=4fc, B=(th_i+1)*g=2ig, C_new=A/2+B=2c_new
                v_s = gact.tile([128, HC * BS], F32, tag="v_s")
                u_s = gact.tile([128, HC * BS], F32, tag="u_s")
                nc.vector.scalar_tensor_tensor(
                    v_s, th[0], 1.0, th[2],
                    op0=mybir.AluOpType.add, op1=mybir.AluOpType.mult)
                nc.vector.scalar_tensor_tensor(
                    u_s, th[1], 1.0, cTf,
                    op0=mybir.AluOpType.add, op1=mybir.AluOpType.mult)
                nc.vector.scalar_tensor_tensor(
                    cTf, u_s, 0.5, v_s,
                    op0=mybir.AluOpType.mult, op1=mybir.AluOpType.add)
                tc_s = gact.tile([128, HC * BS], F32, tag="tc_s")
                nc.scalar.activation(tc_s, cTf, AF.Tanh, scale=0.5)
                # H = (th_o+1)*tanh(c) = 2h
                nc.vector.scalar_tensor_tensor(
                    h_out.rearrange("p c b -> p (c b)"), th[3], 1.0, tc_s,
                    op0=mybir.AluOpType.add, op1=mybir.AluOpType.mult)

                # feedback fc: inT = tanh(0.5*fc(h) + fc_b/2)  [128 i, BS b]
                pf = pfp.tile([128, BS], F32, tag="pf")
                for k in range(HC):
                    nc.tensor.matmul(pf, lhsT=wfc_chunk(k), rhs=h_out[:, k, :],
                                     start=(k == 0), stop=(k == HC - 1))
                nc.scalar.activation(inT, pf, AF.Tanh, bias=fb_h, scale=0.5)

                # output fc in [b, i] layout for clean DMA; bias via K=1 matmul
                po = pop.tile([BS, 128], F32, tag="po")
                for k in range(HC):
                    nc.tensor.matmul(po, lhsT=h_out[:, k, :], rhs=wfc_chunk(k),
                                     start=(k == 0), stop=False)
                nc.tensor.matmul(po, lhsT=ones[:, 0:BS], rhs=fb_r,
                                 start=False, stop=True)
                ob = outp.tile([BS, 128], F16, tag="ob")
                nc.scalar.activation(ob, po, AF.Tanh, scale=0.5)
                # steps past nsteps (timing mode) clamp to row 0 (junk)
                row = smax(0, (nsteps - 1 - t) * BS)
                nc.sync.dma_start(out=out[ds(row, BS), :], in_=ob)

            hint_engines = tuple(getattr(mybir.EngineType, h) for h in hints)
            with tc.For_i(0, n_sv, unroll, staggered_reset=staggered,
                          hint_engines=hint_engines) as t:
                for u in range(0, unroll, 2):
                    step(t + u, hA, hB)
                    step(t + u + 1, hB, hA)

    nc.finalize()
    return nc


def build_pipelined(nsteps: int = T, unroll: int = 8,
                    hints: tuple = ("PE",)):
    """k-major, cross-step software-pipelined variant.

    Engines are in-order, so emission order fixes the per-engine schedule.
    Per step s the PE stream is k-blocks (k=1..4 then 0) of 16 MMs each, with
    step s-1's fc feedback (pf) and output (po) matmuls interleaved between
    blocks -- by the time PE reaches them, the h chunks they need have been
    produced by the ACT/DVE chain running under the k-blocks. The first body
    is peeled (no s-1), and the last step's pf/po/out run in an epilogue.
    """
    U = unroll
    nc = bacc.Bacc()
    cb16 = nc.dram_tensor("cb16", [128, CB_COLS], BF16, kind="ExternalInput")
    cf32 = nc.dram_tensor("cf32", [128, CF_COLS], F32, kind="ExternalInput")
    nit = nc.dram_tensor("niter", [1, 1], mybir.dt.uint32, kind="ExternalInput")
    out = nc.dram_tensor("out", [nsteps * BS, I], F16, kind="ExternalOutput")

    with tile.TileContext(nc) as tc:
        nregs = nc.alloc_registers("niter_regs")
        nc.regs_load(nregs, nit[0:1, 0:1])
        n_sv = nc.snap(nregs, donate=True, min_val=U, max_val=MAX_STEPS)

        with (
            tc.tile_pool(name="consts", bufs=1) as consts,
            tc.tile_pool(name="state", bufs=1) as state,
            tc.tile_pool(name="gact", bufs=3) as gact,
            tc.tile_pool(name="outp", bufs=3) as outp,
            tc.tile_pool(name="psst", bufs=1, space="PSUM") as psst,
            tc.tile_pool(name="pf", bufs=2, space="PSUM") as pfp,
            tc.tile_pool(name="po", bufs=2, space="PSUM") as pop,
        ):
            CB = consts.tile([128, CB_COLS], BF16)
            nc.sync.dma_start(out=CB, in_=cb16[:])
            CF = consts.tile([128, CF_COLS], F32)
            nc.sync.dma_start(out=CF, in_=cf32[:])
            ones = consts.tile([1, HC * BS], BF16)
            nc.vector.memset(ones, 1.0)

            def wg_chunk(m, k):
                o = OFF_WG + (m * NK + k) * 128
                return CB[:, o:o + 128]

            def wfc_chunk(k):
                o = OFF_WFC + k * 128
                return CB[:, o:o + 128]

            fb_r = CB[0:1, OFF_FCBR:OFF_FCBR + 128]
            # bias broadcast laid out [p, h-chunk c, gate j, b]
            BBc = CF[:, OFF_BB:OFF_BB + 4 * HC * BS].rearrange(
                "p (c jb) -> p c jb", c=HC)
            fb_h = CF[:, OFF_FCBH:OFF_FCBH + 1]

            hA = state.tile([128, HC, BS], BF16)
            nc.vector.tensor_copy(
                hA, CB[:, OFF_H0:OFF_H0 + HC * BS].rearrange(
                    "p (c b) -> p c b", c=HC))
            hB = state.tile([128, HC, BS], BF16)
            cT = state.tile([128, HC, BS], F32)
            nc.vector.tensor_copy(
                cT, CF[:, OFF_C0:OFF_C0 + HC * BS].rearrange(
                    "p (c b) -> p c b", c=HC))
            inT = state.tile([128, BS], BF16)
            nc.vector.tensor_copy(inT, CB[:, OFF_XT:OFF_XT + BS])
            warm = state.tile([128, 1], F32)
            nc.scalar.activation(warm, CF[:, OFF_FCBH:OFF_FCBH + 1], AF.Tanh)

            # per-chunk PSUM accumulators [p, gate j, b]
            psc = [psst.tile([128, 4, BS], F32, name=f"psc{c}")
                   for c in range(HC)]
            pscf = [p.rearrange("p j b -> p (j b)") for p in psc]
            for c in range(HC):
                nc.tensor.matmul(pscf[c], lhsT=ones[:, 0:128], rhs=ones,
                                 start=True, stop=True, skip_group_check=True)
                # initial bias pre-write (steady-state ones happen per step)
                nc.vector.tensor_copy(pscf[c], BBc[:, c, :])

            def gates(h_in, prev_tail):
                """k-major gate MMs for one step; prev_tail() emits the
                previous step's pf/po/out between k-blocks."""
                for ki, k in enumerate((1, 2, 3, 4, 0)):
                    for c in range(HC):
                        mv = inT if k == 0 else h_in[:, k - 1, :]
                        for j in range(4):
                            nc.tensor.matmul(
                                psc[c][:, j, :], lhsT=wg_chunk(j * 4 + c, k),
                                rhs=mv, start=False, stop=(k == 0),
                                skip_group_check=True)
                    if prev_tail is not None and ki < 2:
                        prev_tail(ki)

            def tail_for(h_prev, t_prev):
                """pf/po/out for the step that produced h_prev, split in two
                parts to interleave between k-blocks."""
                pf = pfp.tile([128, BS], F32, tag="pf")
                po = pop.tile([BS, 128], F32, tag="po")

                def emit(part):
                    if part == 0:
                        for k in range(HC):
                            nc.tensor.matmul(pf, lhsT=wfc_chunk(k),
                                             rhs=h_prev[:, k, :],
                                             start=(k == 0), stop=(k == HC - 1))
                        nc.scalar.activation(inT, pf, AF.Tanh,
                                             bias=fb_h, scale=0.5)
                    else:
                        for k in range(HC):
                            nc.tensor.matmul(po, lhsT=h_prev[:, k, :],
                                             rhs=wfc_chunk(k),
                                             start=(k == 0), stop=False)
                        nc.tensor.matmul(po, lhsT=ones[:, 0:BS], rhs=fb_r,
                                         start=False, stop=True)
                        ob = outp.tile([BS, 128], F16, tag="ob")
                        nc.scalar.activation(ob, po, AF.Tanh, scale=0.5)
                        row = smax(0, (nsteps - 1 - t_prev) * BS)
                        nc.sync.dma_start(out=out[ds(row, BS), :], in_=ob)
                return emit

            def chain(h_out):
                """per-chunk ACT/DVE: th -> c -> tanh(c) -> h, plus the bias
                re-write for the next step's psum group."""
                for c in range(HC):
                    th_c = gact.tile([128, 4, BS], F32, tag=f"th{c}")
                    nc.scalar.activation(
                        th_c.rearrange("p j b -> p (j b)"), pscf[c],
                        AF.Tanh, scale=0.5)
                    v_c = gact.tile([128, BS], F32, tag=f"v{c}")
                    u_c = gact.tile([128, BS], F32, tag=f"u{c}")
                    # gate order in m-chunks: j=0:i, 1:f, 2:g, 3:o
                    nc.vector.scalar_tensor_tensor(
                        v_c, th_c[:, 0, :], 1.0, th_c[:, 2, :],
                        op0=mybir.AluOpType.add, op1=mybir.AluOpType.mult)
                    nc.vector.scalar_tensor_tensor(
                        u_c, th_c[:, 1, :], 1.0, cT[:, c, :],
                        op0=mybir.AluOpType.add, op1=mybir.AluOpType.mult)
                    nc.vector.scalar_tensor_tensor(
                        cT[:, c, :], u_c, 0.5, v_c,
                        op0=mybir.AluOpType.mult, op1=mybir.AluOpType.add)
                    nc.vector.tensor_copy(pscf[c], BBc[:, c, :])
                    tc_c = gact.tile([128, BS], F32, tag=f"tc{c}")
                    nc.scalar.activation(tc_c, cT[:, c, :], AF.Tanh, scale=0.5)
                    nc.vector.scalar_tensor_tensor(
                        h_out[:, c, :], th_c[:, 3, :], 1.0, tc_c,
                        op0=mybir.AluOpType.add, op1=mybir.AluOpType.mult)

            def body(t, first):
                for u in range(U):
                    h_in = hA if u % 2 == 0 else hB
                    h_out = hB if u % 2 == 0 else hA
                    if first and u == 0:
                        tail = None
                    else:
                        tail = tail_for(h_in, t + u - 1)
                    gates(h_in, tail)
                    chain(h_out)

            body(0, first=True)
            hint_engines = tuple(getattr(mybir.EngineType, h) for h in hints)
            with tc.For_i(U, n_sv, U, staggered_reset=True,
                          hint_engines=hint_engines) as t:
                body(t, first=False)
            # epilogue: last step's feedback/output (h in hA since U even)
            tail = tail_for(hA, n_sv - 1)
            tail(0)
            tail(1)

    nc.finalize()
    return nc


def _prep_core_inputs(x, h0, c0, W_ih, W_hh, b_ih, b_hh, fc_W, fc_b,
                      nsteps=T, niter=T, bias_layout=None):
    if bias_layout is None:
        bias_layout = _STATE.get("bias_layout", "jcb")
    f32 = np.float32
    bf16 = ml_dtypes.bfloat16
    x = np.asarray(x, f32)
    h0 = np.asarray(h0, f32)
    c0 = np.asarray(c0, f32)
    # state is H=2h, C=2c with W_hh/fc_W halved to compensate; g-gate rows
    # doubled so all gates share tanh(0.5*(psum)) with psum pre-biased
    W_cat = np.concatenate(
        [np.asarray(W_ih, f32), 0.5 * np.asarray(W_hh, f32)], axis=1)
    W_cat[1024:1536, :] *= 2.0
    wg_np = W_cat.reshape(NM, 128, NK, 128).transpose(3, 0, 2, 1).reshape(
        128, NM * NK * 128)
    fc_W = np.asarray(fc_W, f32)
    wfc_np = (0.5 * fc_W).reshape(I, HC, 128).transpose(2, 1, 0).reshape(
        128, HC * 128)
    b = np.asarray(b_ih, f32) + np.asarray(b_hh, f32)
    badj = b.copy()
    badj[1024:1536] *= 2.0
    # bias broadcast -> [128, 4*HC*BS]; order (j,c,b) or (c,j,b)
    bb4 = badj.reshape(4, HC, 128).transpose(2, 0, 1)  # [p, j, c]
    if bias_layout == "jcb":
        bb_np = np.broadcast_to(
            bb4[:, :, :, None], (128, 4, HC, BS)).reshape(128, 4 * HC * BS)
    else:
        bb_np = np.broadcast_to(
            bb4.transpose(0, 2, 1)[:, :, :, None],
            (128, HC, 4, BS)).reshape(128, 4 * HC * BS)
    fc_b = np.asarray(fc_b, f32)

    cf = np.zeros((128, CF_COLS), f32)
    cf[:, OFF_BB:OFF_BB + 4 * HC * BS] = bb_np
    cf[:, OFF_FCBH] = 0.5 * fc_b

    cb_common = np.zeros((128, CB_COLS), f32)
    cb_common[:, OFF_WG:OFF_WG + NM * NK * 128] = wg_np
    cb_common[:, OFF_WFC:OFF_WFC + HC * 128] = wfc_np
    cb_common[0, OFF_FCBR:OFF_FCBR + 128] = fc_b

    nit = np.full((1, 1), niter, np.uint32)
    in_maps = []
    for core in range(NCORES):
        sl = slice(core * BS, (core + 1) * BS)
        cb = cb_common.copy()
        cb[:, OFF_XT:OFF_XT + BS] = x[nsteps - 1, sl, :].T
        cb[:, OFF_H0:OFF_H0 + HC * BS] = 2.0 * \
            h0[0, sl, :].reshape(BS, HC, 128).transpose(2, 1, 0).reshape(128, -1)
        cfc = cf.copy()
        cfc[:, OFF_C0:OFF_C0 + HC * BS] = 2.0 * \
            c0[0, sl, :].reshape(BS, HC, 128).transpose(2, 1, 0).reshape(128, -1)
        in_maps.append({
            "cb16": np.ascontiguousarray(cb).astype(bf16),
            "cf32": np.ascontiguousarray(cfc),
            "niter": nit,
        })
    return in_maps


# ---------------------------------------------------------------------------
# PJRT runner: AOT-compiled once, executed per call. Only used under axon
# (the graded environment); native TRN2 falls back to run_bass_kernel_spmd.
# ---------------------------------------------------------------------------

_STATE = {}


def _init_runner():
    if "runner" in _STATE:
        return _STATE["runner"]
    import jax
    import jax.numpy as jnp
    from jax.sharding import Mesh, PartitionSpec, NamedSharding
    import warnings
    with warnings.catch_warnings():
        warnings.simplefilter("ignore")
        from jax.experimental.shard_map import shard_map
    from concourse.bass2jax import (
        _bass_exec_p, install_neuronx_cc_hook, partition_id_tensor)

    install_neuronx_cc_hook()
    nc = _STATE.get("nc")
    if nc is None:
        nc = _STATE["nc"] = build(T)

    partition_name = (nc.partition_id_tensor.name
                      if nc.partition_id_tensor else None)
    in_names, out_names, out_avals = [], [], []
    for alloc in nc.m.functions[0].allocations:
        if not isinstance(alloc, mybir.MemoryLocationSet):
            continue
        name = alloc.memorylocations[0].name
        if alloc.kind == "ExternalInput":
            if name != partition_name:
                in_names.append(name)
        elif alloc.kind == "ExternalOutput":
            out_names.append(name)
            out_avals.append(jax.core.ShapedArray(
                tuple(alloc.tensor_shape), mybir.dt.np(alloc.dtype)))
    n_params = len(in_names)
    n_outs = len(out_avals)
    in_names_all = in_names + out_names + (
        [partition_name] if partition_name else [])

    def _body(*args):
        operands = list(args)
        if partition_name:
            operands.append(partition_id_tensor())
        outs = _bass_exec_p.bind(
            *operands, out_avals=tuple(out_avals),
            in_names=tuple(in_names_all), out_names=tuple(out_names),
            lowering_input_output_aliases=(), sim_require_finite=True,
            sim_require_nnan=True, nc=nc)
        return tuple(outs)

    devs = jax.devices()[:NCORES]
    mesh = Mesh(np.asarray(devs), ("core",))
    donate = tuple(range(n_params, n_params + n_outs))
    sharded = jax.jit(
        shard_map(_body, mesh=mesh,
                  in_specs=(PartitionSpec("core"),) * (n_params + n_outs),
                  out_specs=(PartitionSpec("core"),) * n_outs,
                  check_rep=False),
        donate_argnums=donate, keep_unused=True)

    in_shapes = {
        "cb16": ((NCORES * 128, CB_COLS), ml_dtypes.bfloat16),
        "cf32": ((NCORES * 128, CF_COLS), np.float32),
        "niter": ((NCORES * 1, 1), np.uint32),
    }
    out_shapes = [((NCORES * a.shape[0],) + tuple(a.shape[1:]), a.dtype)
                  for a in out_avals]
    abstract = ([jax.ShapeDtypeStruct(*in_shapes[nm]) for nm in in_names]
                + [jax.ShapeDtypeStruct(s, d) for s, d in out_shapes])
    compiled = sharded.lower(*abstract).compile()

    out_sharding = NamedSharding(mesh, PartitionSpec("core"))
    zeros_fns = [
        jax.jit(lambda s=s, d=d: jnp.zeros(s, d), out_shardings=out_sharding)
        for s, d in out_shapes]

    runner = _STATE["runner"] = {
        "jax": jax, "compiled": compiled, "in_names": in_names,
        "zeros_fns": zeros_fns, "n_outs": n_outs,
    }
    return runner


def _run_axon(in_maps):
    r = _init_runner()
    jax = r["jax"]
    concat_in = [np.concatenate([m[nm] for m in in_maps], axis=0)
                 for nm in r["in_names"]]
    zeros = [fn() for fn in r["zeros_fns"]]
    outs = r["compiled"](*concat_in, *zeros)
    res = np.asarray(outs[0])  # fp16 [NCORES*T*BS, I], one D2H gather
    return res


def warmup():
    """Compile + one tiny dummy exec so later calls are transfer+exec only."""
    if _STATE.get("warm"):
        return
    try:
        r = _init_runner()
        dummy = [{"cb16": np.zeros((128, CB_COLS), ml_dtypes.bfloat16),
                  "cf32": np.zeros((128, CF_COLS), np.float32),
                  "niter": np.full((1, 1), 2, np.uint32)}
                 for _ in range(NCORES)]
        res = _run_axon(dummy)
        del res
        _STATE["warm"] = True
    except Exception:
        import traceback
        traceback.print_exc()


def run(x, h0, c0, W_ih, W_hh, b_ih, b_hh, fc_W, fc_b, niter=T, **kwargs):
    """Returns fp32 [T, B, I] output (valid when niter == T)."""
    in_maps = _prep_core_inputs(x, h0, c0, W_ih, W_hh, b_ih, b_hh, fc_W, fc_b,
                                nsteps=T, niter=niter)
    if axon_active():
        res = _run_axon(in_maps)
        per_core = res.reshape(NCORES, T, BS, I)
    else:
        from concourse.bass_utils import run_bass_kernel_spmd
        nc = _STATE.get("nc")
        if nc is None:
            nc = _STATE["nc"] = build(T)
        rr = run_bass_kernel_spmd(nc, in_maps, core_ids=list(range(NCORES)),
                                  **kwargs)
        per_core = np.stack(
            [r["out"].reshape(T, BS, I) for r in rr.results], axis=0)
    out = np.empty((T, B, I), np.float32)
    for c in range(NCORES):
        out[:, c * BS:(c + 1) * BS, :] = per_core[c]
    return out


def kernel(x, enc_hiddens, h0, c0, W_ih, W_hh, b_ih, b_hh, fc_W, fc_b):
    return run(x, h0, c0, W_ih, W_hh, b_ih, b_hh, fc_W, fc_b)


import os as _os
if axon_active() and not _os.environ.get("KERNEL_NO_WARMUP"):
    warmup()
